# revision 23
# baseline (speedup 1.0000x reference)
"""Trainium2 Bass kernel for the MinGRU (full-GRU) problem.

Shapes (hardcoded): x [256, 512, 256], W*_w [768, 512], W*_b [512],
fc_w [512, 10], fc_b [10].  Output [256, 10] fp32.

Strategy: data-parallel over batch across 8 cores (B_local = 32).

Single interleaved phase per core (v2):
  The input-projection GEMMs (U_g = x @ W_g[H:] + b_g) are interleaved
  into the recurrence's PE idle windows, and U lives entirely in SBUF
  (a 2-chunk ring of 16 timesteps each) -- no DRAM round trip.

  Per step the recurrence keeps everything in a transposed layout
  (partition = H index within a 128-tile, column = 32*k + b):
    - one identity matmul injects u_zr[t] into a [128, 256] PSUM tile
      (cols = 128*g + 32*m + b; g=0 -> z, g=1 -> r),
    - r-gate weight matmuls run first (k-ordered so they can start as
      soon as each half of h lands), then z-gate matmuls,
    - sigmoid of r is split in halves so rh and the h-tilde matmuls
      pipeline with it; the h-tilde/tanh/blend tail is split between
      DVE (first half) and the otherwise-idle Pool engine (second
      half) so both halves of the new h land nearly in parallel,
    - blend is fused: w2 = (z-1)*h via scalar_tensor_tensor, then
      h = z*htilde - w2.
  Epilogue: logits.T = fc_w.T @ h (fp32), written out as [10, 32];
            host transposes and concatenates the 8 core shards.
"""

import os
import sys
import threading

import numpy as np

sys.path.insert(0, "/opt/trn_rl_repo")

import ml_dtypes

BF16 = ml_dtypes.bfloat16

B, T, F, H, C = 256, 512, 256, 512, 10
NCORES = 8
BL = B // NCORES  # 32 batch rows per core
QT = 16           # timesteps per chunk (512 cols = 32 b * 16 t)
NQ = T // QT      # 32 chunks

LAST_EXEC_NS = None

_BUILD_LOCK = threading.Lock()
_CACHED = {}


def _build_bass():
    import concourse.bass as bass
    import concourse.tile as tile
    from concourse import mybir
    from contextlib import ExitStack

    BF = mybir.dt.bfloat16
    F32 = mybir.dt.float32
    AF = mybir.ActivationFunctionType
    ALU = mybir.AluOpType

    nc = bass.Bass()

    # ---- I/O -----------------------------------------------------------
    xT = nc.declare_dram_parameter("xT", [2, NQ, 128, 512], BF, isOutput=False)
    w_rec = nc.declare_dram_parameter("w_rec", [3, 4, 4, 128, 128], BF, isOutput=False)
    w_in = nc.declare_dram_parameter("w_in", [3, 2, 4, 128, 128], BF, isOutput=False)
    bias_gm = nc.declare_dram_parameter("bias_gm", [128, 12], F32, isOutput=False)
    ident = nc.declare_dram_parameter("ident", [128, 128], BF, isOutput=False)
    # Final hidden state in the device layout [p, 32*k + b]; the tiny
    # [512,10] fc projection runs on the host.
    out = nc.declare_dram_parameter("out", [128, 128], BF, isOutput=True)

    with tile.TileContext(nc) as tc, ExitStack() as ctx:
        consts = ctx.enter_context(tc.tile_pool(name="consts", bufs=1))

        # Resident weights / biases.
        wrec_sb = consts.tile([128, 3, 4, 4, 128], BF)
        nc.sync.dma_start(out=wrec_sb, in_=w_rec[:].rearrange("g k m p f -> p g k m f"))
        win_sb = consts.tile([128, 3, 2, 4, 128], BF)
        nc.sync.dma_start(out=win_sb, in_=w_in[:].rearrange("g k m p f -> p g k m f"))
        bias_sb = consts.tile([128, 12], F32)
        nc.sync.dma_start(out=bias_sb, in_=bias_gm[:])
        ident_sb = consts.tile([128, 128], BF)
        nc.sync.dma_start(out=ident_sb, in_=ident[:])

        # Pools.
        xpool = ctx.enter_context(tc.tile_pool(name="xp", bufs=4))
        upool = ctx.enter_context(tc.tile_pool(name="up", bufs=2))
        # PSUM banks (8 total, bank-granular): p1 2 + pra 1 + prb 1 +
        # pz 1 + ph 2 = 7.  The sigmoid tiles are single-buffered: their
        # reads complete ~1.5us before the next step's ident injection.
        p1ps = ctx.enter_context(tc.tile_pool(name="p1ps", bufs=2, space="PSUM"))
        pszr = ctx.enter_context(tc.tile_pool(name="pszr", bufs=1, space="PSUM"))
        pszz = ctx.enter_context(tc.tile_pool(name="pszz", bufs=1, space="PSUM"))
        psh = ctx.enter_context(tc.tile_pool(name="psh", bufs=2, space="PSUM"))
        work = ctx.enter_context(tc.tile_pool(name="work", bufs=2))
        hpool = ctx.enter_context(tc.tile_pool(name="hstate", bufs=1))

        h_sb = hpool.tile([128, 128], BF)
        nc.vector.memset(h_sb, 0.0)

        def load_x(q):
            xa = xpool.tile([128, 512], BF, tag="xa")
            xb = xpool.tile([128, 512], BF, tag="xb")
            nc.sync.dma_start(out=xa, in_=xT[0, q])
            nc.sync.dma_start(out=xb, in_=xT[1, q])
            return xa, xb

        def alloc_u():
            uzr = upool.tile([128, QT, 256], BF, tag="uzr")
            uh = upool.tile([128, QT, 128], BF, tag="uh")
            return uzr, uh

        def emit_unit_mms(g, m, xa, xb):
            """PE half of a (gate, m-tile) input-projection unit: 2 matmuls
            over the F=256 contraction into a PSUM staging tile."""
            ps = p1ps.tile([128, 512], F32, tag="p1")
            nc.tensor.matmul(ps, lhsT=win_sb[:, g, 0, m, :], rhs=xa,
                             start=True, stop=False)
            nc.tensor.matmul(ps, lhsT=win_sb[:, g, 1, m, :], rhs=xb,
                             start=False, stop=True)
            return ps

        def emit_unit_cast(ps, g, m, uzr, uh):
            """Bias-add + bf16 cast of a staged unit into the SBUF u ring.
            u_zr column layout: r block at cols 0:128 (32*m + b), z block
            at cols 128:256.  DVE casts are emitted at the START of the
            following step so they fill the DVE idle window instead of
            delaying the blend in FIFO order."""
            bap = bias_sb[:, g * 4 + m : g * 4 + m + 1]
            psv = ps.rearrange("p (tt b) -> p tt b", b=BL)
            if g < 2:
                base = 128 * (1 - g)  # r first, then z
                dst = uzr.rearrange("p tt (blk b) -> p tt blk b",
                                    blk=8)[:, :, base // 32 + m, :]
                nc.vector.tensor_scalar_add(dst, psv, bap)
            else:
                # Pool/GPSIMD cannot read PSUM; use ACT identity+bias.
                dst = uh.rearrange("p tt (m2 b) -> p tt m2 b",
                                   m2=4)[:, :, m, :]
                nc.scalar.activation(dst, psv, AF.Identity, bias=bap)

        def emit_idents(uzr, uh, tt):
            """Inject u for step (chunk, tt) into fresh psum tiles; returns
            the psum tiles (allocated here so they cycle per step).
            Separate tiles per activation-read granule (r in m01/m23
            halves, z, h-tilde): PSUM RAW deps resolve per accumulation
            group, so each sigmoid fires as soon as its own matmuls end."""
            pra = pszr.tile([128, 64], F32, tag="pra")
            prb = pszr.tile([128, 64], F32, tag="prb")
            pz = pszz.tile([128, 128], F32, tag="pz")
            ph = psh.tile([128, 128], F32, tag="ph")
            nc.tensor.matmul(pra, lhsT=ident_sb, rhs=uzr[:, tt, 0:64],
                             start=True, stop=False, skip_group_check=True)
            nc.tensor.matmul(prb, lhsT=ident_sb, rhs=uzr[:, tt, 64:128],
                             start=True, stop=False, skip_group_check=True)
            nc.tensor.matmul(pz, lhsT=ident_sb, rhs=uzr[:, tt, 128:256],
                             start=True, stop=False, skip_group_check=True)
            nc.tensor.matmul(ph, lhsT=ident_sb, rhs=uh[:, tt],
                             start=True, stop=False, skip_group_check=True)
            return (pra, prb, pz), ph

        def step(pz, ph, units, casts):
            """One recurrence step. pz=(pra, prb, pzz)/ph already hold the
            injected u.  units: (g, m, xa, xb) phase-1 units whose matmuls
            go into this step's PE tail window; casts: staged units from
            the previous step whose DVE/ACT cast runs now.  Returns the
            cast work for the next step."""
            pra, prb, pzz = pz
            # Casts first: they fill the DVE idle window before rh.
            for (ps, g, m, uzr, uh) in casts:
                emit_unit_cast(ps, g, m, uzr, uh)
            zs = work.tile([128, 128], BF, tag="z")
            rs = work.tile([128, 128], BF, tag="r")
            rh = work.tile([128, 128], BF, tag="rh")
            ht = work.tile([128, 128], BF, tag="ht")
            w2 = work.tile([128, 128], BF, tag="w2")
            ea = work.tile([128, 128], BF, tag="e")

            # --- PE: r-gate matmuls first.  k0/k1 need only the first
            # half of h (written early), k2/k3 the second; within each
            # k-pair group, m01 (-> pra) before m23 (-> prb) so sigmoid of
            # the first r half fires as early as possible.
            def r_mm(k, m):
                pg, mm = (pra, m) if m < 2 else (prb, m - 2)
                nc.tensor.matmul(
                    pg[:, 32 * mm : 32 * mm + 32],
                    lhsT=wrec_sb[:, 1, k, m, :],
                    rhs=h_sb[:, 32 * k : 32 * k + 32],
                    start=False, stop=(k == 3),
                    skip_group_check=True)

            for k in (0, 1):
                for m in range(4):
                    r_mm(k, m)
            for m in (0, 1):
                for k in (2, 3):
                    r_mm(k, m)
            for m in (2, 3):
                for k in (2, 3):
                    r_mm(k, m)

            # Phase-1 unit matmuls here: PE idles waiting on sigmoid/rh
            # after the z block anyway, and placing them before the z
            # matmuls makes their DVE cast ready early next step (the
            # engines dispatch ready-first, so a late-ready 750ns cast
            # would otherwise steal DVE right when the blend needs it).
            out_casts = []
            for (g, m, xa, xb, uzr, uh) in units:
                ps = emit_unit_mms(g, m, xa, xb)
                out_casts.append((ps, g, m, uzr, uh))

            # z-gate matmuls (single tile, single sigmoid).
            for k in range(4):
                for m in range(4):
                    nc.tensor.matmul(
                        pzz[:, 32 * m : 32 * m + 32],
                        lhsT=wrec_sb[:, 0, k, m, :],
                        rhs=h_sb[:, 32 * k : 32 * k + 32],
                        start=False, stop=(k == 3),
                        skip_group_check=True)

            # --- ACT: r sigmoid in halves (pipelines rh + h-tilde), then z.
            nc.scalar.activation(rs[:, 0:64], pra, AF.Sigmoid)
            nc.scalar.activation(rs[:, 64:128], prb, AF.Sigmoid)
            nc.scalar.activation(zs, pzz, AF.Sigmoid)

            # --- DVE: rh in halves.
            nc.vector.tensor_mul(rh[:, 0:64], rs[:, 0:64], h_sb[:, 0:64])
            nc.vector.tensor_mul(rh[:, 64:128], rs[:, 64:128], h_sb[:, 64:128])

            # w2 = (z - 1) * h; off the critical path, fused on DVE.
            nc.vector.scalar_tensor_tensor(
                w2, zs, 1.0, h_sb, op0=ALU.subtract, op1=ALU.mult)

            # --- PE: h-tilde matmuls, k-ordered so k0/k1 fire on rh_A.
            for k in range(4):
                for m2 in range(4):
                    nc.tensor.matmul(
                        ph[:, 32 * m2 : 32 * m2 + 32],
                        lhsT=wrec_sb[:, 2, k, m2, :],
                        rhs=rh[:, 32 * k : 32 * k + 32],
                        start=False, stop=(k == 3),
                        skip_group_check=True)

            nc.scalar.activation(ht, ph, AF.Tanh)

            # --- blend: h = z*ht - w2; e full width, h in halves so the
            # next step's k0/k1 matmuls can start on the first half.
            nc.vector.tensor_mul(ea, zs, ht)
            nc.vector.tensor_sub(h_sb[:, 0:64], ea[:, 0:64], w2[:, 0:64])
            nc.vector.tensor_sub(h_sb[:, 64:128], ea[:, 64:128], w2[:, 64:128])

            return out_casts

        # ---- Prologue: chunk 0's U, x prefetches --------------------------
        x_cur = load_x(0)
        u_cur = alloc_u()
        for g in range(3):
            for m in range(4):
                ps = emit_unit_mms(g, m, x_cur[0], x_cur[1])
                emit_unit_cast(ps, g, m, u_cur[0], u_cur[1])
        pz, ph = emit_idents(u_cur[0], u_cur[1], 0)

        # ---- Main loop ----------------------------------------------------
        pending_casts = []
        for q in range(NQ):
            last = q == NQ - 1
            if not last:
                x_next = load_x(q + 1)
                u_next = alloc_u()
                # 12 units spread over steps 2..13.
                sched = {tt: [] for tt in range(QT)}
                for i, (g, m) in enumerate(
                        [(g, m) for g in range(3) for m in range(4)]):
                    sched[2 + i].append(
                        (g, m, x_next[0], x_next[1], u_next[0], u_next[1]))
            else:
                sched = {tt: [] for tt in range(QT)}

            for tt in range(QT):
                cur_pz, cur_ph = pz, ph
                pending_casts = step(cur_pz, cur_ph, sched[tt], pending_casts)
                # inject u for the NEXT step (cycles psum buffers).
                if tt + 1 < QT:
                    pz, ph = emit_idents(u_cur[0], u_cur[1], tt + 1)
                elif not last:
                    pz, ph = emit_idents(u_next[0], u_next[1], 0)

            if not last:
                x_cur, u_cur = x_next, u_next

        # flush any cast left from the final scheduled unit.
        for (ps, g, m, uzr, uh) in pending_casts:
            emit_unit_cast(ps, g, m, uzr, uh)

        # ---- Epilogue: ship the final h; fc runs on the host -----------
        nc.sync.dma_start(out=out[:], in_=h_sb)

    return nc


def _split_multi_waits(nc):
    """Walrus in this container accepts at most ONE embedded sem wait (and
    update) per instruction; Tile emits several.  Split the extras onto
    single-wait NoOps inserted just before (waits) / after (updates) the
    offending instruction on the same engine."""
    from concourse import mybir

    n_split = 0
    for fn in nc.m.functions:
        for blk in fn.blocks:
            insts = blk.instructions
            i = 0
            while i < len(insts):
                ins = insts[i]
                si = ins.sync_info
                if si is None:
                    i += 1
                    continue
                waits = list(si.on_wait)
                updates = list(si.on_update)
                if len(waits) <= 1 and len(updates) <= 1:
                    i += 1
                    continue
                for j, w in enumerate(waits[:-1]):
                    nop = mybir.InstNoOp(
                        name=f"{ins.name}-sw{j}",
                        engine=ins.engine,
                        sync_info=mybir.SyncInfo(on_wait=[w], on_update=[]),
                    )
                    insts.insert(i, nop)
                    i += 1
                for j, u in enumerate(updates[1:]):
                    nop = mybir.InstNoOp(
                        name=f"{ins.name}-su{j}",
                        engine=ins.engine,
                        sync_info=mybir.SyncInfo(on_wait=[], on_update=[u]),
                    )
                    insts.insert(i + 1, nop)
                ins.sync_info = mybir.SyncInfo(
                    on_wait=waits[-1:], on_update=updates[:1])
                n_split += 1
                i += 1 + len(updates[1:])
    return n_split


def _get_nc():
    with _BUILD_LOCK:
        if "nc" not in _CACHED:
            nc = _build_bass()
            _split_multi_waits(nc)
            _CACHED["nc"] = nc
        return _CACHED["nc"]


def _pack_inputs(x, Wz_w, Wz_b, Wr_w, Wr_b, Wh_w, Wh_b, fc_w, fc_b):
    """Host-side layout prep. Returns (shared dict, per-core xT list)."""
    gates_w = [Wz_w, Wr_w, Wh_w]
    gates_b = [Wz_b, Wr_b, Wh_b]

    w_rec = np.stack([
        w[:H].reshape(4, 128, 4, 128).transpose(0, 2, 1, 3) for w in gates_w
    ]).astype(BF16)
    w_in = np.stack([
        w[H:].reshape(2, 128, 4, 128).transpose(0, 2, 1, 3) for w in gates_w
    ]).astype(BF16)
    # bias_gm[p, g*4+m] = b_g[128*m + p]
    bias_gm = np.ascontiguousarray(
        np.stack(gates_b).reshape(3, 4, 128).transpose(2, 0, 1).reshape(128, 12)
    ).astype(np.float32)

    shared = {
        "w_rec": w_rec, "w_in": w_in, "bias_gm": bias_gm,
        "ident": np.eye(128, dtype=BF16),
    }

    xTs = []
    for c in range(NCORES):
        xc = x[c * BL : (c + 1) * BL]  # [32, 512, 256]
        # xT[k, q, p, 32*tt + b] = xc[b, 16*q + tt, 128*k + p]
        arr = xc.reshape(BL, NQ, QT, 2, 128).transpose(3, 1, 4, 2, 0)
        xTs.append(np.ascontiguousarray(arr.reshape(2, NQ, 128, 512)).astype(BF16))
    return shared, xTs


def kernel(x, Wz_w, Wz_b, Wr_w, Wr_b, Wh_w, Wh_b, fc_w, fc_b):
    global LAST_EXEC_NS
    from concourse.bass_utils import run_bass_kernel_spmd

    x = np.asarray(x, dtype=np.float32)
    shared, xTs = _pack_inputs(
        x, np.asarray(Wz_w), np.asarray(Wz_b), np.asarray(Wr_w),
        np.asarray(Wr_b), np.asarray(Wh_w), np.asarray(Wh_b),
        np.asarray(fc_w), np.asarray(fc_b))

    nc = _get_nc()
    in_maps = [dict(shared, xT=xTs[c]) for c in range(NCORES)]
    trace = bool(int(os.environ.get("GRU_TRACE", "0")))
    res = run_bass_kernel_spmd(nc, in_maps, list(range(NCORES)), trace=trace)
    LAST_EXEC_NS = res.exec_time_ns

    # out is h.T in device layout: out[p, 32*k + b] = h[b, 128*k + p].
    fc_w32 = np.asarray(fc_w, dtype=np.float32)
    fc_b32 = np.asarray(fc_b, dtype=np.float32)
    logits = []
    for c in range(NCORES):
        hT = np.asarray(res.results[c]["out"], dtype=np.float32)  # [128,128]
        h_c = hT.reshape(128, 4, BL).transpose(2, 1, 0).reshape(BL, H)
        logits.append(h_c @ fc_w32 + fc_b32)
    return np.concatenate(logits, axis=0).astype(np.float32)


# revision 26
# speedup vs baseline: 1.2252x; 1.2252x over previous
"""Trainium2 Bass kernel for the MinGRU (full-GRU) problem.

Shapes (hardcoded): x [256, 512, 256], W*_w [768, 512], W*_b [512],
fc_w [512, 10], fc_b [10].  Output [256, 10] fp32.

Strategy: data-parallel over batch across 8 cores (B_local = 32).

Single interleaved phase per core (v2):
  The input-projection GEMMs (U_g = x @ W_g[H:] + b_g) are interleaved
  into the recurrence's PE idle windows, and U lives entirely in SBUF
  (a 2-chunk ring of 16 timesteps each) -- no DRAM round trip.

  Per step the recurrence keeps everything in a transposed layout
  (partition = H index within a 128-tile, column = 32*k + b):
    - one identity matmul injects u_zr[t] into a [128, 256] PSUM tile
      (cols = 128*g + 32*m + b; g=0 -> z, g=1 -> r),
    - r-gate weight matmuls run first (k-ordered so they can start as
      soon as each half of h lands), then z-gate matmuls,
    - sigmoid of r is split in halves so rh and the h-tilde matmuls
      pipeline with it; the h-tilde/tanh/blend tail is split between
      DVE (first half) and the otherwise-idle Pool engine (second
      half) so both halves of the new h land nearly in parallel,
    - blend is fused: w2 = (z-1)*h via scalar_tensor_tensor, then
      h = z*htilde - w2.
  Epilogue: logits.T = fc_w.T @ h (fp32), written out as [10, 32];
            host transposes and concatenates the 8 core shards.
"""

import os
import sys
import threading

import numpy as np

sys.path.insert(0, "/opt/trn_rl_repo")

import ml_dtypes

BF16 = ml_dtypes.bfloat16

B, T, F, H, C = 256, 512, 256, 512, 10
NCORES = 8
BL = B // NCORES  # 32 batch rows per core
QT = 16           # timesteps per chunk (512 cols = 32 b * 16 t)
NQ = T // QT      # 32 chunks

LAST_EXEC_NS = None

_BUILD_LOCK = threading.Lock()
_CACHED = {}


def _build_bass():
    import concourse.bass as bass
    import concourse.tile as tile
    from concourse import mybir
    from contextlib import ExitStack

    BF = mybir.dt.bfloat16
    F32 = mybir.dt.float32
    AF = mybir.ActivationFunctionType
    ALU = mybir.AluOpType

    nc = bass.Bass()

    # ---- I/O -----------------------------------------------------------
    xT = nc.declare_dram_parameter("xT", [2, NQ, 128, 512], BF, isOutput=False)
    w_rec = nc.declare_dram_parameter("w_rec", [3, 4, 4, 128, 128], BF, isOutput=False)
    w_in = nc.declare_dram_parameter("w_in", [3, 2, 4, 128, 128], BF, isOutput=False)
    bias_gm = nc.declare_dram_parameter("bias_gm", [128, 12], F32, isOutput=False)
    ident = nc.declare_dram_parameter("ident", [128, 128], BF, isOutput=False)
    # Final hidden state in the device layout [p, 32*k + b]; the tiny
    # [512,10] fc projection runs on the host.
    out = nc.declare_dram_parameter("out", [128, 128], BF, isOutput=True)

    with tile.TileContext(nc) as tc, ExitStack() as ctx:
        consts = ctx.enter_context(tc.tile_pool(name="consts", bufs=1))

        # Resident weights / biases.
        wrec_sb = consts.tile([128, 3, 4, 4, 128], BF)
        nc.sync.dma_start(out=wrec_sb, in_=w_rec[:].rearrange("g k m p f -> p g k m f"))
        win_sb = consts.tile([128, 3, 2, 4, 128], BF)
        nc.sync.dma_start(out=win_sb, in_=w_in[:].rearrange("g k m p f -> p g k m f"))
        bias_sb = consts.tile([128, 12], F32)
        nc.sync.dma_start(out=bias_sb, in_=bias_gm[:])
        ident_sb = consts.tile([128, 128], BF)
        nc.sync.dma_start(out=ident_sb, in_=ident[:])

        # Pools.
        xpool = ctx.enter_context(tc.tile_pool(name="xp", bufs=4))
        upool = ctx.enter_context(tc.tile_pool(name="up", bufs=2))
        # PSUM banks (8 total, bank-granular): p1 2 + pra 1 + prb 1 +
        # pz 1 + ph 2 = 7.  The sigmoid tiles are single-buffered: their
        # reads complete ~1.5us before the next step's ident injection.
        p1ps = ctx.enter_context(tc.tile_pool(name="p1ps", bufs=2, space="PSUM"))
        pszr = ctx.enter_context(tc.tile_pool(name="pszr", bufs=1, space="PSUM"))
        pszz = ctx.enter_context(tc.tile_pool(name="pszz", bufs=1, space="PSUM"))
        psh = ctx.enter_context(tc.tile_pool(name="psh", bufs=2, space="PSUM"))
        work = ctx.enter_context(tc.tile_pool(name="work", bufs=2))
        hpool = ctx.enter_context(tc.tile_pool(name="hstate", bufs=1))

        h_sb = hpool.tile([128, 128], BF)
        nc.vector.memset(h_sb, 0.0)

        def load_x(q):
            xa = xpool.tile([128, 512], BF, tag="xa")
            xb = xpool.tile([128, 512], BF, tag="xb")
            nc.sync.dma_start(out=xa, in_=xT[0, q])
            nc.sync.dma_start(out=xb, in_=xT[1, q])
            return xa, xb

        def alloc_u():
            uzr = upool.tile([128, QT, 256], BF, tag="uzr")
            uh = upool.tile([128, QT, 128], BF, tag="uh")
            return uzr, uh

        def emit_unit_mms(g, m, xa, xb):
            """PE half of a (gate, m-tile) input-projection unit: 2 matmuls
            over the F=256 contraction into a PSUM staging tile."""
            ps = p1ps.tile([128, 512], F32, tag="p1")
            nc.tensor.matmul(ps, lhsT=win_sb[:, g, 0, m, :], rhs=xa,
                             start=True, stop=False)
            nc.tensor.matmul(ps, lhsT=win_sb[:, g, 1, m, :], rhs=xb,
                             start=False, stop=True)
            return ps

        def emit_unit_cast(ps, g, m, uzr, uh):
            """Bias-add + bf16 cast of a staged unit into the SBUF u ring.
            u_zr column layout: r block at cols 0:128 (32*m + b), z block
            at cols 128:256.  DVE casts are emitted at the START of the
            following step so they fill the DVE idle window instead of
            delaying the blend in FIFO order."""
            bap = bias_sb[:, g * 4 + m : g * 4 + m + 1]
            psv = ps.rearrange("p (tt b) -> p tt b", b=BL)
            if g < 2:
                base = 128 * (1 - g)  # r first, then z
                dst = uzr.rearrange("p tt (blk b) -> p tt blk b",
                                    blk=8)[:, :, base // 32 + m, :]
                # Quarter the cast (~230ns each): engines dispatch
                # ready-first, so no single long op can straddle the
                # moment the blend ops become ready.
                for qt4 in range(4):
                    nc.vector.tensor_scalar_add(
                        dst[:, 4 * qt4 : 4 * qt4 + 4, :],
                        psv[:, 4 * qt4 : 4 * qt4 + 4, :], bap)
            else:
                # Pool/GPSIMD cannot read PSUM; use ACT identity+bias.
                dst = uh.rearrange("p tt (m2 b) -> p tt m2 b",
                                   m2=4)[:, :, m, :]
                nc.scalar.activation(dst, psv, AF.Identity, bias=bap)

        def emit_idents(uzr, uh, tt):
            """Inject u for step (chunk, tt) into fresh psum tiles; returns
            the psum tiles (allocated here so they cycle per step).
            Separate tiles per activation-read granule (r in m01/m23
            halves, z, h-tilde): PSUM RAW deps resolve per accumulation
            group, so each sigmoid fires as soon as its own matmuls end."""
            pra = pszr.tile([128, 64], F32, tag="pra")
            prb = pszr.tile([128, 64], F32, tag="prb")
            pz = pszz.tile([128, 128], F32, tag="pz")
            ph = psh.tile([128, 128], F32, tag="ph")
            nc.tensor.matmul(pra, lhsT=ident_sb, rhs=uzr[:, tt, 0:64],
                             start=True, stop=False, skip_group_check=True)
            nc.tensor.matmul(prb, lhsT=ident_sb, rhs=uzr[:, tt, 64:128],
                             start=True, stop=False, skip_group_check=True)
            nc.tensor.matmul(pz, lhsT=ident_sb, rhs=uzr[:, tt, 128:256],
                             start=True, stop=False, skip_group_check=True)
            nc.tensor.matmul(ph, lhsT=ident_sb, rhs=uh[:, tt],
                             start=True, stop=False, skip_group_check=True)
            return (pra, prb, pz), ph

        def step(pz, ph, units, casts):
            """One recurrence step. pz=(pra, prb, pzz)/ph already hold the
            injected u.  units: (g, m, xa, xb) phase-1 units whose matmuls
            go into this step's PE tail window; casts: staged units from
            the previous step whose DVE/ACT cast runs now.  Returns the
            cast work for the next step."""
            pra, prb, pzz = pz
            # Casts first: they fill the DVE idle window before rh.
            for (ps, g, m, uzr, uh) in casts:
                emit_unit_cast(ps, g, m, uzr, uh)
            zs = work.tile([128, 128], BF, tag="z")
            rs = work.tile([128, 128], BF, tag="r")
            rh = work.tile([128, 128], BF, tag="rh")
            ht = work.tile([128, 128], BF, tag="ht")
            w2 = work.tile([128, 128], BF, tag="w2")
            ea = work.tile([128, 128], BF, tag="e")

            # --- PE: r-gate matmuls first.  k0/k1 need only the first
            # half of h (written early), k2/k3 the second; within each
            # k-pair group, m01 (-> pra) before m23 (-> prb) so sigmoid of
            # the first r half fires as early as possible.
            def r_mm(k, m):
                pg, mm = (pra, m) if m < 2 else (prb, m - 2)
                nc.tensor.matmul(
                    pg[:, 32 * mm : 32 * mm + 32],
                    lhsT=wrec_sb[:, 1, k, m, :],
                    rhs=h_sb[:, 32 * k : 32 * k + 32],
                    start=False, stop=(k == 3),
                    skip_group_check=True)

            for k in (0, 1):
                for m in range(4):
                    r_mm(k, m)
            for m in (0, 1):
                for k in (2, 3):
                    r_mm(k, m)
            for m in (2, 3):
                for k in (2, 3):
                    r_mm(k, m)

            # z-gate matmuls (single tile, single sigmoid).
            for k in range(4):
                for m in range(4):
                    nc.tensor.matmul(
                        pzz[:, 32 * m : 32 * m + 32],
                        lhsT=wrec_sb[:, 0, k, m, :],
                        rhs=h_sb[:, 32 * k : 32 * k + 32],
                        start=False, stop=(k == 3),
                        skip_group_check=True)

            # --- ACT: r sigmoid in halves (pipelines rh + h-tilde), then z.
            nc.scalar.activation(rs[:, 0:64], pra, AF.Sigmoid)
            nc.scalar.activation(rs[:, 64:128], prb, AF.Sigmoid)
            nc.scalar.activation(zs, pzz, AF.Sigmoid)

            # --- DVE: rh in halves.
            nc.vector.tensor_mul(rh[:, 0:64], rs[:, 0:64], h_sb[:, 0:64])
            nc.vector.tensor_mul(rh[:, 64:128], rs[:, 64:128], h_sb[:, 64:128])

            # w2 = (z - 1) * h; off the critical path, fused on DVE.
            nc.vector.scalar_tensor_tensor(
                w2, zs, 1.0, h_sb, op0=ALU.subtract, op1=ALU.mult)

            # --- PE: h-tilde matmuls, k-ordered so k0/k1 fire on rh_A.
            for k in range(4):
                for m2 in range(4):
                    nc.tensor.matmul(
                        ph[:, 32 * m2 : 32 * m2 + 32],
                        lhsT=wrec_sb[:, 2, k, m2, :],
                        rhs=rh[:, 32 * k : 32 * k + 32],
                        start=False, stop=(k == 3),
                        skip_group_check=True)

            nc.scalar.activation(ht, ph, AF.Tanh)

            # --- blend: h = z*ht - w2; e full width, h in halves so the
            # next step's k0/k1 matmuls can start on the first half.
            nc.vector.tensor_mul(ea, zs, ht)
            nc.vector.tensor_sub(h_sb[:, 0:64], ea[:, 0:64], w2[:, 0:64])
            nc.vector.tensor_sub(h_sb[:, 64:128], ea[:, 64:128], w2[:, 64:128])

            # --- PE tail window: phase-1 unit matmuls for the next chunk;
            # their casts are returned to run early next step.
            out_casts = []
            for (g, m, xa, xb, uzr, uh) in units:
                ps = emit_unit_mms(g, m, xa, xb)
                out_casts.append((ps, g, m, uzr, uh))
            return out_casts

        # ---- Prologue: chunk 0's U, x prefetches --------------------------
        x_cur = load_x(0)
        u_cur = alloc_u()
        for g in range(3):
            for m in range(4):
                ps = emit_unit_mms(g, m, x_cur[0], x_cur[1])
                emit_unit_cast(ps, g, m, u_cur[0], u_cur[1])
        pz, ph = emit_idents(u_cur[0], u_cur[1], 0)

        # ---- Main loop ----------------------------------------------------
        pending_casts = []
        for q in range(NQ):
            last = q == NQ - 1
            if not last:
                x_next = load_x(q + 1)
                u_next = alloc_u()
                # 12 units spread over steps 2..13.
                sched = {tt: [] for tt in range(QT)}
                for i, (g, m) in enumerate(
                        [(g, m) for g in range(3) for m in range(4)]):
                    sched[2 + i].append(
                        (g, m, x_next[0], x_next[1], u_next[0], u_next[1]))
            else:
                sched = {tt: [] for tt in range(QT)}

            for tt in range(QT):
                cur_pz, cur_ph = pz, ph
                pending_casts = step(cur_pz, cur_ph, sched[tt], pending_casts)
                # inject u for the NEXT step (cycles psum buffers).
                if tt + 1 < QT:
                    pz, ph = emit_idents(u_cur[0], u_cur[1], tt + 1)
                elif not last:
                    pz, ph = emit_idents(u_next[0], u_next[1], 0)

            if not last:
                x_cur, u_cur = x_next, u_next

        # flush any cast left from the final scheduled unit.
        for (ps, g, m, uzr, uh) in pending_casts:
            emit_unit_cast(ps, g, m, uzr, uh)

        # ---- Epilogue: ship the final h; fc runs on the host -----------
        nc.sync.dma_start(out=out[:], in_=h_sb)

    return nc


def _split_multi_waits(nc):
    """Walrus in this container accepts at most ONE embedded sem wait (and
    update) per instruction; Tile emits several.  Split the extras onto
    single-wait NoOps inserted just before (waits) / after (updates) the
    offending instruction on the same engine."""
    from concourse import mybir

    n_split = 0
    for fn in nc.m.functions:
        for blk in fn.blocks:
            insts = blk.instructions
            i = 0
            while i < len(insts):
                ins = insts[i]
                si = ins.sync_info
                if si is None:
                    i += 1
                    continue
                waits = list(si.on_wait)
                updates = list(si.on_update)
                if len(waits) <= 1 and len(updates) <= 1:
                    i += 1
                    continue
                for j, w in enumerate(waits[:-1]):
                    nop = mybir.InstNoOp(
                        name=f"{ins.name}-sw{j}",
                        engine=ins.engine,
                        sync_info=mybir.SyncInfo(on_wait=[w], on_update=[]),
                    )
                    insts.insert(i, nop)
                    i += 1
                for j, u in enumerate(updates[1:]):
                    nop = mybir.InstNoOp(
                        name=f"{ins.name}-su{j}",
                        engine=ins.engine,
                        sync_info=mybir.SyncInfo(on_wait=[], on_update=[u]),
                    )
                    insts.insert(i + 1, nop)
                ins.sync_info = mybir.SyncInfo(
                    on_wait=waits[-1:], on_update=updates[:1])
                n_split += 1
                i += 1 + len(updates[1:])
    return n_split


def _get_nc():
    with _BUILD_LOCK:
        if "nc" not in _CACHED:
            nc = _build_bass()
            _split_multi_waits(nc)
            _CACHED["nc"] = nc
        return _CACHED["nc"]


def _pack_inputs(x, Wz_w, Wz_b, Wr_w, Wr_b, Wh_w, Wh_b, fc_w, fc_b):
    """Host-side layout prep. Returns (shared dict, per-core xT list)."""
    gates_w = [Wz_w, Wr_w, Wh_w]
    gates_b = [Wz_b, Wr_b, Wh_b]

    w_rec = np.stack([
        w[:H].reshape(4, 128, 4, 128).transpose(0, 2, 1, 3) for w in gates_w
    ]).astype(BF16)
    w_in = np.stack([
        w[H:].reshape(2, 128, 4, 128).transpose(0, 2, 1, 3) for w in gates_w
    ]).astype(BF16)
    # bias_gm[p, g*4+m] = b_g[128*m + p]
    bias_gm = np.ascontiguousarray(
        np.stack(gates_b).reshape(3, 4, 128).transpose(2, 0, 1).reshape(128, 12)
    ).astype(np.float32)

    shared = {
        "w_rec": w_rec, "w_in": w_in, "bias_gm": bias_gm,
        "ident": np.eye(128, dtype=BF16),
    }

    xTs = []
    for c in range(NCORES):
        xc = x[c * BL : (c + 1) * BL]  # [32, 512, 256]
        # xT[k, q, p, 32*tt + b] = xc[b, 16*q + tt, 128*k + p]
        arr = xc.reshape(BL, NQ, QT, 2, 128).transpose(3, 1, 4, 2, 0)
        xTs.append(np.ascontiguousarray(arr.reshape(2, NQ, 128, 512)).astype(BF16))
    return shared, xTs


def kernel(x, Wz_w, Wz_b, Wr_w, Wr_b, Wh_w, Wh_b, fc_w, fc_b):
    global LAST_EXEC_NS
    from concourse.bass_utils import run_bass_kernel_spmd

    x = np.asarray(x, dtype=np.float32)
    shared, xTs = _pack_inputs(
        x, np.asarray(Wz_w), np.asarray(Wz_b), np.asarray(Wr_w),
        np.asarray(Wr_b), np.asarray(Wh_w), np.asarray(Wh_b),
        np.asarray(fc_w), np.asarray(fc_b))

    nc = _get_nc()
    in_maps = [dict(shared, xT=xTs[c]) for c in range(NCORES)]
    trace = bool(int(os.environ.get("GRU_TRACE", "0")))
    res = run_bass_kernel_spmd(nc, in_maps, list(range(NCORES)), trace=trace)
    LAST_EXEC_NS = res.exec_time_ns

    # out is h.T in device layout: out[p, 32*k + b] = h[b, 128*k + p].
    fc_w32 = np.asarray(fc_w, dtype=np.float32)
    fc_b32 = np.asarray(fc_b, dtype=np.float32)
    logits = []
    for c in range(NCORES):
        hT = np.asarray(res.results[c]["out"], dtype=np.float32)  # [128,128]
        h_c = hT.reshape(128, 4, BL).transpose(2, 1, 0).reshape(BL, H)
        logits.append(h_c @ fc_w32 + fc_b32)
    return np.concatenate(logits, axis=0).astype(np.float32)


# revision 27
# speedup vs baseline: 1.2254x; 1.0002x over previous
"""Trainium2 Bass kernel for the MinGRU (full-GRU) problem.

Shapes (hardcoded): x [256, 512, 256], W*_w [768, 512], W*_b [512],
fc_w [512, 10], fc_b [10].  Output [256, 10] fp32.

Strategy: data-parallel over batch across 8 cores (B_local = 32).

Single interleaved phase per core (v2):
  The input-projection GEMMs (U_g = x @ W_g[H:] + b_g) are interleaved
  into the recurrence's PE idle windows, and U lives entirely in SBUF
  (a 2-chunk ring of 16 timesteps each) -- no DRAM round trip.

  Per step the recurrence keeps everything in a transposed layout
  (partition = H index within a 128-tile, column = 32*k + b):
    - one identity matmul injects u_zr[t] into a [128, 256] PSUM tile
      (cols = 128*g + 32*m + b; g=0 -> z, g=1 -> r),
    - r-gate weight matmuls run first (k-ordered so they can start as
      soon as each half of h lands), then z-gate matmuls,
    - sigmoid of r is split in halves so rh and the h-tilde matmuls
      pipeline with it; the h-tilde/tanh/blend tail is split between
      DVE (first half) and the otherwise-idle Pool engine (second
      half) so both halves of the new h land nearly in parallel,
    - blend is fused: w2 = (z-1)*h via scalar_tensor_tensor, then
      h = z*htilde - w2.
  Epilogue: logits.T = fc_w.T @ h (fp32), written out as [10, 32];
            host transposes and concatenates the 8 core shards.
"""

import os
import sys
import threading

import numpy as np

sys.path.insert(0, "/opt/trn_rl_repo")

import ml_dtypes

BF16 = ml_dtypes.bfloat16

B, T, F, H, C = 256, 512, 256, 512, 10
NCORES = 8
BL = B // NCORES  # 32 batch rows per core
QT = 16           # timesteps per chunk (512 cols = 32 b * 16 t)
NQ = T // QT      # 32 chunks

LAST_EXEC_NS = None

_BUILD_LOCK = threading.Lock()
_CACHED = {}


def _build_bass():
    import concourse.bass as bass
    import concourse.tile as tile
    from concourse import mybir
    from contextlib import ExitStack

    BF = mybir.dt.bfloat16
    F32 = mybir.dt.float32
    AF = mybir.ActivationFunctionType
    ALU = mybir.AluOpType

    nc = bass.Bass()

    # ---- I/O -----------------------------------------------------------
    xT = nc.declare_dram_parameter("xT", [2, NQ, 128, 512], BF, isOutput=False)
    w_rec = nc.declare_dram_parameter("w_rec", [3, 4, 4, 128, 128], BF, isOutput=False)
    w_in = nc.declare_dram_parameter("w_in", [3, 2, 4, 128, 128], BF, isOutput=False)
    bias_gm = nc.declare_dram_parameter("bias_gm", [128, 12], F32, isOutput=False)
    ident = nc.declare_dram_parameter("ident", [128, 128], BF, isOutput=False)
    # Final hidden state in the device layout [p, 32*k + b]; the tiny
    # [512,10] fc projection runs on the host.
    out = nc.declare_dram_parameter("out", [128, 128], BF, isOutput=True)

    with tile.TileContext(nc) as tc, ExitStack() as ctx:
        consts = ctx.enter_context(tc.tile_pool(name="consts", bufs=1))

        # Resident weights / biases.
        wrec_sb = consts.tile([128, 3, 4, 4, 128], BF)
        nc.sync.dma_start(out=wrec_sb, in_=w_rec[:].rearrange("g k m p f -> p g k m f"))
        win_sb = consts.tile([128, 3, 2, 4, 128], BF)
        nc.sync.dma_start(out=win_sb, in_=w_in[:].rearrange("g k m p f -> p g k m f"))
        bias_sb = consts.tile([128, 12], F32)
        nc.sync.dma_start(out=bias_sb, in_=bias_gm[:])
        ident_sb = consts.tile([128, 128], BF)
        nc.sync.dma_start(out=ident_sb, in_=ident[:])

        # Pools.
        xpool = ctx.enter_context(tc.tile_pool(name="xp", bufs=4))
        upool = ctx.enter_context(tc.tile_pool(name="up", bufs=2))
        # PSUM banks (8 total, bank-granular): p1 2 + pra 1 + prb 1 +
        # pz 1 + ph 2 = 7.  The sigmoid tiles are single-buffered: their
        # reads complete ~1.5us before the next step's ident injection.
        p1ps = ctx.enter_context(tc.tile_pool(name="p1ps", bufs=2, space="PSUM"))
        pszr = ctx.enter_context(tc.tile_pool(name="pszr", bufs=1, space="PSUM"))
        pszz = ctx.enter_context(tc.tile_pool(name="pszz", bufs=1, space="PSUM"))
        psh = ctx.enter_context(tc.tile_pool(name="psh", bufs=2, space="PSUM"))
        work = ctx.enter_context(tc.tile_pool(name="work", bufs=2))
        hpool = ctx.enter_context(tc.tile_pool(name="hstate", bufs=1))

        h_sb = hpool.tile([128, 128], BF)
        nc.vector.memset(h_sb, 0.0)

        def load_x(q):
            xa = xpool.tile([128, 512], BF, tag="xa")
            xb = xpool.tile([128, 512], BF, tag="xb")
            nc.sync.dma_start(out=xa, in_=xT[0, q])
            nc.sync.dma_start(out=xb, in_=xT[1, q])
            return xa, xb

        def alloc_u():
            uzr = upool.tile([128, QT, 256], BF, tag="uzr")
            uh = upool.tile([128, QT, 128], BF, tag="uh")
            return uzr, uh

        def emit_unit_mms(g, m, xa, xb):
            """PE half of a (gate, m-tile) input-projection unit: 2 matmuls
            over the F=256 contraction into a PSUM staging tile."""
            ps = p1ps.tile([128, 512], F32, tag="p1")
            nc.tensor.matmul(ps, lhsT=win_sb[:, g, 0, m, :], rhs=xa,
                             start=True, stop=False)
            nc.tensor.matmul(ps, lhsT=win_sb[:, g, 1, m, :], rhs=xb,
                             start=False, stop=True)
            return ps

        def emit_unit_cast(ps, g, m, uzr, uh):
            """Bias-add + bf16 cast of a staged unit into the SBUF u ring.
            u_zr column layout: r block at cols 0:128 (32*m + b), z block
            at cols 128:256.  DVE casts are emitted at the START of the
            following step so they fill the DVE idle window instead of
            delaying the blend in FIFO order."""
            bap = bias_sb[:, g * 4 + m : g * 4 + m + 1]
            psv = ps.rearrange("p (tt b) -> p tt b", b=BL)
            if g < 2:
                base = 128 * (1 - g)  # r first, then z
                dst = uzr.rearrange("p tt (blk b) -> p tt blk b",
                                    blk=8)[:, :, base // 32 + m, :]
                # Quarter the cast (~230ns each): engines dispatch
                # ready-first, so no single long op can straddle the
                # moment the blend ops become ready.
                for qt4 in range(4):
                    nc.vector.tensor_scalar_add(
                        dst[:, 4 * qt4 : 4 * qt4 + 4, :],
                        psv[:, 4 * qt4 : 4 * qt4 + 4, :], bap)
            else:
                # Pool/GPSIMD cannot read PSUM; use ACT identity+bias.
                dst = uh.rearrange("p tt (m2 b) -> p tt m2 b",
                                   m2=4)[:, :, m, :]
                nc.scalar.activation(dst, psv, AF.Identity, bias=bap)

        def emit_idents(uzr, uh, tt):
            """Inject u for step (chunk, tt) into fresh psum tiles; returns
            the psum tiles (allocated here so they cycle per step).
            Separate tiles per activation-read granule (r in m01/m23
            halves, z, h-tilde): PSUM RAW deps resolve per accumulation
            group, so each sigmoid fires as soon as its own matmuls end."""
            pra = pszr.tile([128, 64], F32, tag="pra")
            prb = pszr.tile([128, 64], F32, tag="prb")
            pz = pszz.tile([128, 128], F32, tag="pz")
            ph = psh.tile([128, 128], F32, tag="ph")
            nc.tensor.matmul(pra, lhsT=ident_sb, rhs=uzr[:, tt, 0:64],
                             start=True, stop=False, skip_group_check=True)
            nc.tensor.matmul(prb, lhsT=ident_sb, rhs=uzr[:, tt, 64:128],
                             start=True, stop=False, skip_group_check=True)
            nc.tensor.matmul(pz, lhsT=ident_sb, rhs=uzr[:, tt, 128:256],
                             start=True, stop=False, skip_group_check=True)
            nc.tensor.matmul(ph, lhsT=ident_sb, rhs=uh[:, tt],
                             start=True, stop=False, skip_group_check=True)
            return (pra, prb, pz), ph

        def step(pz, ph, units, casts):
            """One recurrence step. pz=(pra, prb, pzz)/ph already hold the
            injected u.  units: (g, m, xa, xb) phase-1 units whose matmuls
            go into this step's PE tail window; casts: staged units from
            the previous step whose DVE/ACT cast runs now.  Returns the
            cast work for the next step."""
            pra, prb, pzz = pz
            # Casts first: they fill the DVE idle window before rh.
            for (ps, g, m, uzr, uh) in casts:
                emit_unit_cast(ps, g, m, uzr, uh)
            zs = work.tile([128, 128], BF, tag="z")
            rs = work.tile([128, 128], BF, tag="r")
            rh = work.tile([128, 128], BF, tag="rh")
            ht = work.tile([128, 128], BF, tag="ht")
            w2 = work.tile([128, 128], BF, tag="w2")
            ea = work.tile([128, 128], BF, tag="e")

            # --- PE: r-gate matmuls first.  k0/k1 need only the first
            # half of h (written early), k2/k3 the second; within each
            # k-pair group, m01 (-> pra) before m23 (-> prb) so sigmoid of
            # the first r half fires as early as possible.
            def r_mm(k, m):
                pg, mm = (pra, m) if m < 2 else (prb, m - 2)
                nc.tensor.matmul(
                    pg[:, 32 * mm : 32 * mm + 32],
                    lhsT=wrec_sb[:, 1, k, m, :],
                    rhs=h_sb[:, 32 * k : 32 * k + 32],
                    start=False, stop=(k == 3),
                    skip_group_check=True)

            for k in (0, 1):
                for m in range(4):
                    r_mm(k, m)
            for m in (0, 1):
                for k in (2, 3):
                    r_mm(k, m)
            for m in (2, 3):
                for k in (2, 3):
                    r_mm(k, m)

            # z-gate matmuls (single tile, single sigmoid).
            for k in range(4):
                for m in range(4):
                    nc.tensor.matmul(
                        pzz[:, 32 * m : 32 * m + 32],
                        lhsT=wrec_sb[:, 0, k, m, :],
                        rhs=h_sb[:, 32 * k : 32 * k + 32],
                        start=False, stop=(k == 3),
                        skip_group_check=True)

            # --- ACT: r sigmoid in halves (pipelines rh + h-tilde), then z.
            nc.scalar.activation(rs[:, 0:64], pra, AF.Sigmoid)
            nc.scalar.activation(rs[:, 64:128], prb, AF.Sigmoid)
            nc.scalar.activation(zs, pzz, AF.Sigmoid)

            # --- DVE: rh in halves.
            nc.vector.tensor_mul(rh[:, 0:64], rs[:, 0:64], h_sb[:, 0:64])
            nc.vector.tensor_mul(rh[:, 64:128], rs[:, 64:128], h_sb[:, 64:128])

            # w2 = (z - 1) * h; off the critical path, fused on DVE.
            nc.vector.scalar_tensor_tensor(
                w2, zs, 1.0, h_sb, op0=ALU.subtract, op1=ALU.mult)

            # --- PE: h-tilde matmuls, k-ordered so k0/k1 fire on rh_A.
            for k in range(4):
                for m2 in range(4):
                    nc.tensor.matmul(
                        ph[:, 32 * m2 : 32 * m2 + 32],
                        lhsT=wrec_sb[:, 2, k, m2, :],
                        rhs=rh[:, 32 * k : 32 * k + 32],
                        start=False, stop=(k == 3),
                        skip_group_check=True)

            nc.scalar.activation(ht, ph, AF.Tanh)

            # --- blend: h = z*ht - w2; e full width, h in halves so the
            # next step's k0/k1 matmuls can start on the first half.
            nc.vector.tensor_mul(ea, zs, ht)
            nc.vector.tensor_sub(h_sb[:, 0:64], ea[:, 0:64], w2[:, 0:64])
            nc.vector.tensor_sub(h_sb[:, 64:128], ea[:, 64:128], w2[:, 64:128])

            # --- PE tail window: phase-1 unit matmuls for the next chunk;
            # their casts are returned to run early next step.
            out_casts = []
            for (g, m, xa, xb, uzr, uh) in units:
                ps = emit_unit_mms(g, m, xa, xb)
                out_casts.append((ps, g, m, uzr, uh))
            return out_casts

        # ---- Prologue: chunk 0's U, x prefetches --------------------------
        x_cur = load_x(0)
        u_cur = alloc_u()
        for g in range(3):
            for m in range(4):
                ps = emit_unit_mms(g, m, x_cur[0], x_cur[1])
                emit_unit_cast(ps, g, m, u_cur[0], u_cur[1])
        pz, ph = emit_idents(u_cur[0], u_cur[1], 0)

        # ---- Main loop ----------------------------------------------------
        pending_casts = []
        for q in range(NQ):
            last = q == NQ - 1
            if not last:
                x_next = load_x(q + 1)
                u_next = alloc_u()
                # 12 units spread over steps 0..11, keeping the chunk
                # boundary steps (14,15,0..3 showed +300-600ns) unit-free.
                sched = {tt: [] for tt in range(QT)}
                for i, (g, m) in enumerate(
                        [(g, m) for g in range(3) for m in range(4)]):
                    sched[i].append(
                        (g, m, x_next[0], x_next[1], u_next[0], u_next[1]))
            else:
                sched = {tt: [] for tt in range(QT)}

            for tt in range(QT):
                cur_pz, cur_ph = pz, ph
                pending_casts = step(cur_pz, cur_ph, sched[tt], pending_casts)
                # inject u for the NEXT step (cycles psum buffers).
                if tt + 1 < QT:
                    pz, ph = emit_idents(u_cur[0], u_cur[1], tt + 1)
                elif not last:
                    pz, ph = emit_idents(u_next[0], u_next[1], 0)

            if not last:
                x_cur, u_cur = x_next, u_next

        # flush any cast left from the final scheduled unit.
        for (ps, g, m, uzr, uh) in pending_casts:
            emit_unit_cast(ps, g, m, uzr, uh)

        # ---- Epilogue: ship the final h; fc runs on the host -----------
        nc.sync.dma_start(out=out[:], in_=h_sb)

    return nc


def _split_multi_waits(nc):
    """Walrus in this container accepts at most ONE embedded sem wait (and
    update) per instruction; Tile emits several.  Split the extras onto
    single-wait NoOps inserted just before (waits) / after (updates) the
    offending instruction on the same engine."""
    from concourse import mybir

    n_split = 0
    for fn in nc.m.functions:
        for blk in fn.blocks:
            insts = blk.instructions
            i = 0
            while i < len(insts):
                ins = insts[i]
                si = ins.sync_info
                if si is None:
                    i += 1
                    continue
                waits = list(si.on_wait)
                updates = list(si.on_update)
                if len(waits) <= 1 and len(updates) <= 1:
                    i += 1
                    continue
                for j, w in enumerate(waits[:-1]):
                    nop = mybir.InstNoOp(
                        name=f"{ins.name}-sw{j}",
                        engine=ins.engine,
                        sync_info=mybir.SyncInfo(on_wait=[w], on_update=[]),
                    )
                    insts.insert(i, nop)
                    i += 1
                for j, u in enumerate(updates[1:]):
                    nop = mybir.InstNoOp(
                        name=f"{ins.name}-su{j}",
                        engine=ins.engine,
                        sync_info=mybir.SyncInfo(on_wait=[], on_update=[u]),
                    )
                    insts.insert(i + 1, nop)
                ins.sync_info = mybir.SyncInfo(
                    on_wait=waits[-1:], on_update=updates[:1])
                n_split += 1
                i += 1 + len(updates[1:])
    return n_split


def _get_nc():
    with _BUILD_LOCK:
        if "nc" not in _CACHED:
            nc = _build_bass()
            _split_multi_waits(nc)
            _CACHED["nc"] = nc
        return _CACHED["nc"]


def _pack_inputs(x, Wz_w, Wz_b, Wr_w, Wr_b, Wh_w, Wh_b, fc_w, fc_b):
    """Host-side layout prep. Returns (shared dict, per-core xT list)."""
    gates_w = [Wz_w, Wr_w, Wh_w]
    gates_b = [Wz_b, Wr_b, Wh_b]

    w_rec = np.stack([
        w[:H].reshape(4, 128, 4, 128).transpose(0, 2, 1, 3) for w in gates_w
    ]).astype(BF16)
    w_in = np.stack([
        w[H:].reshape(2, 128, 4, 128).transpose(0, 2, 1, 3) for w in gates_w
    ]).astype(BF16)
    # bias_gm[p, g*4+m] = b_g[128*m + p]
    bias_gm = np.ascontiguousarray(
        np.stack(gates_b).reshape(3, 4, 128).transpose(2, 0, 1).reshape(128, 12)
    ).astype(np.float32)

    shared = {
        "w_rec": w_rec, "w_in": w_in, "bias_gm": bias_gm,
        "ident": np.eye(128, dtype=BF16),
    }

    xTs = []
    for c in range(NCORES):
        xc = x[c * BL : (c + 1) * BL]  # [32, 512, 256]
        # xT[k, q, p, 32*tt + b] = xc[b, 16*q + tt, 128*k + p]
        arr = xc.reshape(BL, NQ, QT, 2, 128).transpose(3, 1, 4, 2, 0)
        xTs.append(np.ascontiguousarray(arr.reshape(2, NQ, 128, 512)).astype(BF16))
    return shared, xTs


def kernel(x, Wz_w, Wz_b, Wr_w, Wr_b, Wh_w, Wh_b, fc_w, fc_b):
    global LAST_EXEC_NS
    from concourse.bass_utils import run_bass_kernel_spmd

    x = np.asarray(x, dtype=np.float32)
    shared, xTs = _pack_inputs(
        x, np.asarray(Wz_w), np.asarray(Wz_b), np.asarray(Wr_w),
        np.asarray(Wr_b), np.asarray(Wh_w), np.asarray(Wh_b),
        np.asarray(fc_w), np.asarray(fc_b))

    nc = _get_nc()
    in_maps = [dict(shared, xT=xTs[c]) for c in range(NCORES)]
    trace = bool(int(os.environ.get("GRU_TRACE", "0")))
    res = run_bass_kernel_spmd(nc, in_maps, list(range(NCORES)), trace=trace)
    LAST_EXEC_NS = res.exec_time_ns

    # out is h.T in device layout: out[p, 32*k + b] = h[b, 128*k + p].
    fc_w32 = np.asarray(fc_w, dtype=np.float32)
    fc_b32 = np.asarray(fc_b, dtype=np.float32)
    logits = []
    for c in range(NCORES):
        hT = np.asarray(res.results[c]["out"], dtype=np.float32)  # [128,128]
        h_c = hT.reshape(128, 4, BL).transpose(2, 1, 0).reshape(BL, H)
        logits.append(h_c @ fc_w32 + fc_b32)
    return np.concatenate(logits, axis=0).astype(np.float32)
